# revision 16
# baseline (speedup 1.0000x reference)
"""Trainium2 Bass kernel for AdaptiveCantorModalityFusion.

Strategy: data-parallel over batch across 8 NeuronCores (2 batches/core,
weights replicated, no collectives). On-chip pipeline per core:

  x -> (dma transpose, feature-major) -> p = x@Wp + bp  -> gate MLP ->
  z = p * (a*gate + 1-a)  -> qkv = z@Wqkv + (emb@Wqkv + bqkv)  ->
  pairwise 2-way softmax attention (clip_l<->t5_l, clip_g<->t5_g) ->
  out = ctx@Wout  (token-major, direct DMA out)

The reference's 4-source masked softmax collapses to a 2-way softmax:
w_self = sigmoid((d_self - d_cross)/c - beta_pair). Padded positions of
the short (clip) modalities contribute K=bk=0 / V=bv=0, so for t5 target
positions s>=77 the cross score is 0 and the partner V vanishes.

Compute dtype bf16 (f32 PSUM accumulation); weights/activations are cast
to bf16 on host to halve DMA traffic. Output f32.
"""

import numpy as np
import ml_dtypes

B, S, D, H, HD, M = 16, 256, 1024, 16, 64, 4
DIMS = [768, 1280, 2048, 2048]
SEQS = [77, 77, 256, 256]
NCORES = 8
BL = B // NCORES                    # 2 batches per core
TOKS = [BL * s for s in SEQS]       # [154, 154, 512, 512]
KCH = [d // 128 for d in DIMS]      # [6, 10, 16, 16]
OUT_OFF = [0, 77, 154, 410]
TOTSEQ = sum(SEQS)                  # 666
NQC = 3 * D // 128                  # 24 qkv output chunks
PAIRS = [(0, 2), (1, 3)]

BF16 = ml_dtypes.bfloat16

_cache = {}


def _build(cinv, nbeta, a_gate):
    """Build the per-core Bass program. cinv/nbeta/a_gate are python floats
    baked into the instruction stream (they come from scalar inputs)."""
    import sys
    if '/opt/trn_rl_repo' not in sys.path:
        sys.path.insert(0, '/opt/trn_rl_repo')
    import concourse.bass as bass
    import concourse.mybir as mybir
    from concourse import bacc
    from concourse.tile import TileContext

    dt = mybir.dt
    AF = mybir.ActivationFunctionType

    nc = bacc.Bacc("TRN2", target_bir_lowering=False, debug=False,
                   num_devices=NCORES)

    # ---- DRAM parameters ----
    xp = [nc.declare_dram_parameter(f"x{m}", [DIMS[m], TOKS[m]], dt.bfloat16,
                                    isOutput=False) for m in range(M)]
    wp = [nc.declare_dram_parameter(f"wp{m}", [DIMS[m], D], dt.bfloat16,
                                    isOutput=False) for m in range(M)]
    wg1 = nc.declare_dram_parameter("wg1", [M * D, D // 4], dt.bfloat16, isOutput=False)
    wg2r = nc.declare_dram_parameter("wg2r", [M * (D // 4), 128], dt.bfloat16, isOutput=False)
    wqkv = nc.declare_dram_parameter("wqkv", [D, 3 * D], dt.bfloat16, isOutput=False)
    wout = nc.declare_dram_parameter("wout", [M * D, D], dt.bfloat16, isOutput=False)
    bp = nc.declare_dram_parameter("bp", [M * 128, 8], dt.float32, isOutput=False)
    bg1 = nc.declare_dram_parameter("bg1", [M * 128, 2], dt.float32, isOutput=False)
    bg2 = nc.declare_dram_parameter("bg2", [M * 128, 1], dt.float32, isOutput=False)
    bqkv = nc.declare_dram_parameter("bqkv", [M * 128, NQC], dt.float32, isOutput=False)
    seg = nc.declare_dram_parameter("seg", [8 * 128, 16], dt.bfloat16, isOutput=False)
    segt = nc.declare_dram_parameter("segt", [8 * 128, 128], dt.bfloat16, isOutput=False)
    out = nc.declare_dram_parameter("out", [BL * TOTSEQ, D], dt.float32, isOutput=True)

    with TileContext(nc) as tc:
        with tc.tile_pool(name="const", bufs=1) as constp, \
             tc.tile_pool(name="psum", bufs=8, space="PSUM") as psump, \
             tc.tile_pool(name="pz", bufs=1) as pzp, \
             tc.tile_pool(name="qkv", bufs=1) as qkvp:

            # constants
            bp_t = constp.tile([128, M, 8], dt.float32, tag="bp")
            nc.sync.dma_start(bp_t[:], bp.ap().rearrange("(m p) c -> p m c", p=128))
            bg1_t = constp.tile([128, M, 2], dt.float32, tag="bg1")
            nc.sync.dma_start(bg1_t[:], bg1.ap().rearrange("(m p) c -> p m c", p=128))
            bg2_t = constp.tile([128, M, 1], dt.float32, tag="bg2")
            nc.sync.dma_start(bg2_t[:], bg2.ap().rearrange("(m p) c -> p m c", p=128))
            bqkv_t = constp.tile([128, M, NQC], dt.float32, tag="bqkv")
            nc.sync.dma_start(bqkv_t[:], bqkv.ap().rearrange("(m p) c -> p m c", p=128))
            seg_t = constp.tile([128, 8, 16], dt.bfloat16, tag="seg")
            nc.sync.dma_start(seg_t[:], seg.ap().rearrange("(k p) c -> p k c", p=128))
            segt_t = constp.tile([128, 8, 128], dt.bfloat16, tag="segt")
            nc.sync.dma_start(segt_t[:], segt.ap().rearrange("(k p) c -> p k c", p=128))
            nb_t = constp.tile([128, 2], dt.float32, tag="nb")
            nc.vector.memset(nb_t[:, 0:1], float(nbeta[0]))
            nc.vector.memset(nb_t[:, 1:2], float(nbeta[1]))

            zt = {}     # feature-major gated activations per modality
            qk = {}     # feature-major q/k/v per modality

            # prefetch Wqkv while stages A-C run (pool opened before theirs so
            # its region never overlaps their freed space -> no false dep)
            wqkvp_cm = tc.tile_pool(name="wqkvp", bufs=1)
            wqkvp = wqkvp_cm.__enter__()
            wq_t = wqkvp.tile([128, 8, 3 * D], dt.bfloat16, tag="wqkv")

            # ---- stages A-C: load/transpose x, project, gate ----
            with tc.tile_pool(name="xt", bufs=2) as xtp, \
                 tc.tile_pool(name="wpp", bufs=6) as wpp, \
                 tc.tile_pool(name="wgp", bufs=2) as wgp, \
                 tc.tile_pool(name="gt", bufs=2) as gtp:
                for mi, m in enumerate([2, 0, 3, 1]):
                    if mi == 2:
                        # issue the Wqkv prefetch once the early-phase DMAs
                        # are in flight; it completes during m3/m1 + gates
                        nc.sync.dma_start(wq_t[:], wqkv.ap()
                                          .rearrange("(k p) n -> p k n", p=128))
                    T, KC = TOKS[m], KCH[m]
                    # A: x arrives host-transposed [d, T]; plain strided DMA
                    xt_m = xtp.tile([128, KC, T], dt.bfloat16, tag="xt")
                    nc.sync.dma_start(out=xt_m[:],
                                      in_=xp[m].ap().rearrange("(k p) t -> p k t", p=128))
                    # B: p.T = Wp.T @ x.T + bp   (feature-major [128, 8, T])
                    # Wp streamed in per-kc chunks (keeps SBUF small, deep
                    # prefetch via bufs)
                    wpin = wp[m].ap().rearrange("(k p) n -> p k n", p=128)
                    p_ps = [psump.tile([128, 512], dt.float32, tag="bank", name="ppsum")[:, :T]
                            for _ in range(8)]
                    for kc in range(KC):
                        wp_k = wpp.tile([128, D], dt.bfloat16, tag="wpc", name="wpk")
                        nc.sync.dma_start(wp_k[:], wpin[:, kc, :])
                        for mc in range(8):
                            nc.tensor.matmul(p_ps[mc], wp_k[:, mc * 128:(mc + 1) * 128],
                                             xt_m[:, kc, :],
                                             start=(kc == 0), stop=(kc == KC - 1))
                    pz_m = pzp.tile([128, 8, T], dt.bfloat16, tag=f"pz{m}")
                    for mc in range(8):
                        nc.scalar.add(pz_m[:, mc, :], p_ps[mc], bp_t[:, m, mc:mc + 1])

                    # C: gate MLP on p, then z = p * (a*sig + (1-a))
                    wg1_m = wgp.tile([128, 8, 256], dt.bfloat16, tag="wg1")
                    nc.sync.dma_start(wg1_m[:], wg1.ap()[m * D:(m + 1) * D, :]
                                      .rearrange("(k p) n -> p k n", p=128))
                    h_ps = [psump.tile([128, 512], dt.float32, tag="bank", name="hpsum")[:, :T]
                            for _ in range(2)]
                    for kc in range(8):
                        for hc in range(2):
                            nc.tensor.matmul(h_ps[hc], wg1_m[:, kc, hc * 128:(hc + 1) * 128],
                                             pz_m[:, kc, :],
                                             start=(kc == 0), stop=(kc == 7))
                    h_t = gtp.tile([128, 2, 512], dt.bfloat16, tag="h", name="ht")[:, :, :T]
                    for hc in range(2):
                        nc.scalar.activation(h_t[:, hc, :], h_ps[hc], AF.Gelu,
                                             bias=bg1_t[:, m, hc:hc + 1])
                    wg2_m = wgp.tile([128, 2, 128], dt.bfloat16, tag="wg2")
                    nc.sync.dma_start(wg2_m[:], wg2r.ap()[m * 256:(m + 1) * 256, :]
                                      .rearrange("(k p) n -> p k n", p=128))
                    g_ps = psump.tile([128, 512], dt.float32, tag="bank", name="gpsum")[:, :T]
                    for hc in range(2):
                        nc.tensor.matmul(g_ps, wg2_m[:, hc, :], h_t[:, hc, :],
                                         start=(hc == 0), stop=(hc == 1))
                    sg = gtp.tile([128, 512], dt.float32, tag="sg", name="sg")[:, :T]
                    nc.scalar.activation(sg, g_ps, AF.Sigmoid, bias=bg2_t[:, m, 0:1])
                    sc = gtp.tile([128, 512], dt.bfloat16, tag="sc", name="sc")[:, :T]
                    nc.vector.tensor_scalar(sc, sg, float(a_gate[m]),
                                            float(1.0 - a_gate[m]),
                                            mybir.AluOpType.mult, mybir.AluOpType.add)
                    for mc in range(8):
                        nc.vector.tensor_mul(pz_m[:, mc, :], pz_m[:, mc, :], sc)
                    zt[m] = pz_m

            # ---- stages D-F, pair-pipelined ----
            # PE stream: qkv(p0) qkv(p1) | scores+rep(p0) wout(m2,m0) |
            #            scores+rep(p1) wout(m3,m1); DVE does prods/ctx of a
            # pair while PE works on the other pair / wout.
            def qkv_stage(mods, wq_t):
                for m in mods:
                    qk[m] = qkvp.tile([128, NQC, TOKS[m]], dt.bfloat16,
                                      tag=f"qk{m}", name=f"qk{m}")
                for oc in range(NQC):
                    q_ps = {m: psump.tile([128, 512], dt.float32, tag="bank",
                                          name="qpsum")[:, :TOKS[m]] for m in mods}
                    for kc in range(8):
                        for m in mods:
                            nc.tensor.matmul(q_ps[m], wq_t[:, kc, oc * 128:(oc + 1) * 128],
                                             zt[m][:, kc, :],
                                             start=(kc == 0), stop=(kc == 7))
                    for m in mods:
                        nc.scalar.add(qk[m][:, oc, :], q_ps[m], bqkv_t[:, m, oc:oc + 1])

            def aview(ap3):
                return ap3.rearrange("p (b s) -> p b s", b=BL)

            def bviewv(ap3, SA):
                return ap3.rearrange("p (b s) -> p b s", b=BL)[:, :, :SA]

            def attn_scores(pi, atp, awp):
                A, Bm = PAIRS[pi]
                TA, TB, SA = TOKS[A], TOKS[Bm], SEQS[A]
                qA, qB = qk[A], qk[Bm]
                dA_ps = psump.tile([128, 512], dt.float32, tag="bank", name="dApsum")[:16, :TA]
                dB_ps = psump.tile([128, 512], dt.float32, tag="bank", name="dBpsum")[:16, :TB]
                for kc in range(8):
                    pAA = atp.tile([128, 154], dt.bfloat16, tag="pa")
                    nc.vector.tensor_mul(pAA, qA[:, kc, :], qA[:, 8 + kc, :])
                    nc.tensor.matmul(dA_ps, seg_t[:, kc, :], pAA,
                                     start=(kc == 0), stop=False)
                    pAB = atp.tile([128, 154], dt.bfloat16, tag="pa")
                    nc.vector.scalar_tensor_tensor(
                        aview(pAB), aview(qA[:, kc, :]), -1.0,
                        bviewv(qB[:, 8 + kc, :], SA),
                        mybir.AluOpType.mult, mybir.AluOpType.mult)
                    nc.tensor.matmul(dA_ps, seg_t[:, kc, :], pAB,
                                     start=False, stop=(kc == 7))
                    # dB: full-width self matmul opens (kc==0) / closes (kc==7)
                    # the accumulation group; subset cross matmuls sit inside.
                    pBB = atp.tile([128, 512], dt.bfloat16, tag="pb")
                    nc.vector.tensor_mul(pBB, qB[:, kc, :], qB[:, 8 + kc, :])
                    pBA = atp.tile([128, 154], dt.bfloat16, tag="pa")
                    nc.vector.scalar_tensor_tensor(
                        aview(pBA), bviewv(qB[:, kc, :], SA), -1.0,
                        aview(qA[:, 8 + kc, :]),
                        mybir.AluOpType.mult, mybir.AluOpType.mult)
                    dBv = dB_ps.rearrange("h (b s) -> h b s", b=BL)[:, :, :SA]
                    if kc == 0:
                        nc.tensor.matmul(dB_ps, seg_t[:, kc, :], pBB,
                                         start=True, stop=False)
                        nc.tensor.matmul(dBv, seg_t[:, kc, :], aview(pBA),
                                         start=False, stop=False, skip_group_check=True)
                    else:
                        nc.tensor.matmul(dBv, seg_t[:, kc, :], aview(pBA),
                                         start=False, stop=False, skip_group_check=True)
                        nc.tensor.matmul(dB_ps, seg_t[:, kc, :], pBB,
                                         start=False, stop=(kc == 7))
                # sigmoid weights; rows 16..127 zeroed for the K=128 broadcast
                wA_t = awp.tile([128, 154], dt.bfloat16, tag="wa")
                nc.vector.memset(wA_t[:], 0.0)
                nc.scalar.activation(wA_t[:16, :], dA_ps, AF.Sigmoid,
                                     bias=nb_t[:16, pi:pi + 1], scale=float(cinv))
                wB_t = awp.tile([128, 512], dt.bfloat16, tag="wb")
                nc.vector.memset(wB_t[:], 0.0)
                nc.scalar.activation(wB_t[:16, :], dB_ps, AF.Sigmoid,
                                     scale=float(cinv))
                # broadcast weights across the 64-row head blocks via PE
                reps = []
                for kc in range(8):
                    rA_ps = psump.tile([128, 512], dt.float32, tag="bank", name="rApsum")[:, :TA]
                    nc.tensor.matmul(rA_ps, segt_t[:, kc, :], wA_t, start=True, stop=True)
                    rA = awp.tile([128, 154], dt.bfloat16, tag=f"ra{kc}", name="ra")
                    nc.vector.tensor_copy(rA, rA_ps)
                    rB_ps = psump.tile([128, 512], dt.float32, tag="bank", name="rBpsum")[:, :TB]
                    nc.tensor.matmul(rB_ps, segt_t[:, kc, :], wB_t, start=True, stop=True)
                    rB = awp.tile([128, 512], dt.bfloat16, tag=f"rb{kc}", name="rb")
                    nc.vector.tensor_copy(rB, rB_ps)
                    reps.append((rA, rB))
                return reps

            def attn_ctx_b(pi, reps, atp):
                A, Bm = PAIRS[pi]
                SA = SEQS[A]
                qA, qB = qk[A], qk[Bm]
                for kc in range(8):
                    rA, rB = reps[kc]
                    # ctxB = wB*vB everywhere; += vA - wB*vA on valid cols
                    t2 = atp.tile([128, 154], dt.bfloat16, tag="pa")
                    nc.vector.tensor_mul(aview(t2), bviewv(rB, SA),
                                         aview(qA[:, 16 + kc, :]))
                    t3 = atp.tile([128, 154], dt.bfloat16, tag="pa")
                    nc.vector.tensor_sub(t3, qA[:, 16 + kc, :], t2)
                    nc.vector.tensor_mul(qB[:, kc, :], rB, qB[:, 16 + kc, :])
                    nc.vector.tensor_add(bviewv(qB[:, kc, :], SA),
                                         bviewv(qB[:, kc, :], SA), aview(t3))

            def attn_ctx_a(pi, reps, atp):
                A, Bm = PAIRS[pi]
                SA = SEQS[A]
                qA, qB = qk[A], qk[Bm]
                for kc in range(8):
                    rA, rB = reps[kc]
                    # ctxA = wA*(vA - vB) + vB   (over the Q chunks)
                    t1 = atp.tile([128, 154], dt.bfloat16, tag="pa")
                    nc.vector.tensor_sub(aview(t1), aview(qA[:, 16 + kc, :]),
                                         bviewv(qB[:, 16 + kc, :], SA))
                    nc.vector.tensor_mul(t1, t1, rA)
                    nc.vector.tensor_add(aview(qA[:, kc, :]), aview(t1),
                                         bviewv(qB[:, 16 + kc, :], SA))

            def wout_stage(m, wo_m, outp):
                T = TOKS[m]
                for tci in range((T + 127) // 128):
                    t0 = tci * 128
                    tcs = min(128, T - t0)
                    o_ps = [psump.tile([128, 512], dt.float32, tag="bank",
                                       name="opsum")[:tcs, :] for _ in range(2)]
                    for kc in range(8):
                        for nh in range(2):
                            nc.tensor.matmul(o_ps[nh], qk[m][:, kc, t0:t0 + tcs],
                                             wo_m[:, kc, nh * 512:(nh + 1) * 512],
                                             start=(kc == 0), stop=(kc == 7))
                    o_sb = outp.tile([128, D], dt.float32, tag="ot", name="osb")[:tcs, :]
                    for nh in range(2):
                        nc.scalar.copy(o_sb[:, nh * 512:(nh + 1) * 512], o_ps[nh])
                    r = 0
                    while r < tcs:   # <=2 contiguous (batch, seq) runs
                        tok = t0 + r
                        b, s = divmod(tok, SEQS[m])
                        run = min(tcs - r, SEQS[m] - s)
                        orow = b * TOTSEQ + OUT_OFF[m] + s
                        nc.sync.dma_start(out=out.ap()[orow:orow + run, :],
                                          in_=o_sb[r:r + run, :])
                        r += run

            qkv_stage([0, 2], wq_t)
            qkv_stage([1, 3], wq_t)
            wqkvp_cm.__exit__(None, None, None)

            with tc.tile_pool(name="woutp", bufs=1) as wop, \
                 tc.tile_pool(name="outp", bufs=3) as outp, \
                 tc.tile_pool(name="attn", bufs=4) as atp, \
                 tc.tile_pool(name="attw", bufs=2) as awp:
                wo = {}
                for m in range(M):   # prefetch all Wout while attention runs
                    wo[m] = wop.tile([128, 8, D], dt.bfloat16, tag=f"wo{m}", name=f"wo{m}")
                    nc.sync.dma_start(wo[m][:], wout.ap()[m * D:(m + 1) * D, :]
                                      .rearrange("(k p) n -> p k n", p=128))
                reps0 = attn_scores(0, atp, awp)
                attn_ctx_b(0, reps0, atp)
                wout_stage(2, wo[2], outp)
                attn_ctx_a(0, reps0, atp)
                wout_stage(0, wo[0], outp)
                reps1 = attn_scores(1, atp, awp)
                attn_ctx_b(1, reps1, atp)
                wout_stage(3, wo[3], outp)
                attn_ctx_a(1, reps1, atp)
                wout_stage(1, wo[1], outp)
    nc.compile()
    return nc


def _prep(inputs):
    """Host-side preprocessing: bf16 casts, bias folding, layout prep."""
    f32 = np.float32
    names = ["clip_l", "clip_g", "t5_l", "t5_g"]
    W = {k: np.asarray(v) for k, v in inputs.items()}

    temp = float(np.abs(W["temperature"]))
    cinv = 1.0 / (np.sqrt(HD) * temp)
    betas = np.asarray(W["betas"], f32)
    nbeta = [-float(betas[0]), -float(betas[1])]
    a_gate = [float(1.0 / (1.0 + np.exp(-W["alphas"][m]))) for m in range(M)]

    wqkv = np.concatenate([W["Wq"], W["Wk"], W["Wv"]], axis=1).astype(f32)
    emb = W["emb"].astype(f32)
    bqkv_full = emb @ wqkv + np.concatenate([W["bq"], W["bk"], W["bv"]])[None, :]
    # shared weights map (same for every core)
    shared = {
        "wg1": W["Wg1"].reshape(M * D, D // 4).astype(BF16),
        "wg2r": np.repeat(W["Wg2"].reshape(M * (D // 4), 1), 128, axis=1).astype(BF16),
        "wqkv": wqkv.astype(BF16),
        "wout": W["Wout"].reshape(M * D, D).astype(BF16),
        "bqkv": bqkv_full.astype(f32).reshape(M, NQC, 128).transpose(0, 2, 1)
                 .reshape(M * 128, NQC).copy(),
        "bg2": np.repeat(W["bg2"].reshape(M, 1), 128, axis=1).reshape(M * 128, 1)
                 .astype(f32).copy(),
    }
    for m, nm in enumerate(names):
        shared[f"wp{m}"] = W[f"Wp_{nm}"].astype(BF16)
    bp_all, bg1_all = [], []
    for m, nm in enumerate(names):
        bp_all.append(W[f"bp_{nm}"].astype(f32).reshape(8, 128).T)
        bg1_all.append(W["bg1"][m].astype(f32).reshape(2, 128).T)
    shared["bp"] = np.concatenate(bp_all, axis=0).copy()
    shared["bg1"] = np.concatenate(bg1_all, axis=0).copy()

    segv = np.zeros((8, 128, 16), f32)
    segtv = np.zeros((8, 128, 128), f32)
    for kc in range(8):
        for j in range(128):
            h = 2 * kc + j // 64
            segv[kc, j, h] = 1.0
            segtv[kc, h, j] = 1.0
    shared["seg"] = segv.reshape(8 * 128, 16).astype(BF16)
    shared["segt"] = segtv.reshape(8 * 128, 128).astype(BF16)

    in_maps = []
    for c in range(NCORES):
        im = dict(shared)
        for m, nm in enumerate(names):
            xs = np.asarray(W[f"x_{nm}"])[c * BL:(c + 1) * BL].reshape(TOKS[m], DIMS[m])
            im[f"x{m}"] = np.ascontiguousarray(xs.T).astype(BF16)
        in_maps.append(im)
    return in_maps, cinv, nbeta, a_gate


def kernel(**inputs):
    import sys
    if '/opt/trn_rl_repo' not in sys.path:
        sys.path.insert(0, '/opt/trn_rl_repo')
    from concourse.bass_utils import run_bass_kernel_spmd

    in_maps, cinv, nbeta, a_gate = _prep(inputs)
    key = (round(cinv, 9), round(nbeta[0], 9), round(nbeta[1], 9),
           tuple(round(a, 9) for a in a_gate))
    if key not in _cache:
        _cache[key] = _build(cinv, nbeta, a_gate)
    nc = _cache[key]

    res = run_bass_kernel_spmd(nc, in_maps, list(range(NCORES)))
    outs = [res.results[c]["out"].reshape(BL, TOTSEQ, D) for c in range(NCORES)]
    full = np.concatenate(outs, axis=0).astype(np.float32)
    # bout is additive at the very end; apply on host (exact)
    bout = np.asarray(inputs["bout"], np.float32)
    for m in range(M):
        sl = slice(OUT_OFF[m], OUT_OFF[m] + SEQS[m])
        full[:, sl, :] += bout[m][None, None, :]
    return full
